# revision 15
# baseline (speedup 1.0000x reference)
"""CMSA (cross-modal self-attention) Trainium2 Bass kernel, v15.

Problem: two feature maps x,y of [B=4, C=256, H=64, W=64]. Per sample:
  q_y,k_y = 1x1conv(y) -> [32, N]; v_x = 1x1conv(x) -> [256, N]  (N=4096)
  att_y = softmax(q_y^T k_y); enhanced_x = v_x @ att_y^T + x
  (and symmetrically x->y). Output: (enhanced_x, enhanced_y).

Sharding: 8 independent attention problems = (4 samples) x (2 directions),
one per NeuronCore, SPMD. Per-core kernel computes one full attention.

Kernel math (per core):
  L^T[j,i] = sum_d k[d,j] q[d,i]     (k-tile stationary fp16)
  U^T[j,i] = exp(L^T[j,i])           (unnormalized bf16)
  T[i, 0:256] = sum_j U^T[j,i] V^T[j,c]   "transposed AV": U^T-slice is the
  T[i, 256]   = sum_j U^T[j,i]             stationary operand, [V^T | ones]
                                           (257 cols) is the moving operand;
                                           denominator rides as column 256
  out^T[i,c] = T[i,c] / T[i,256] + (feat_v^T[i,c] + bv[c])

v8 schedule. The ACT exp stream (1.104us per [128,1024] tile, 128 tiles
= 141us) is the pacer; the PE carries ~9% more work than ACT, so the
structural excess surfaces only in the tail:
  - DMA: sync queue sends fv chunks 0-5 first (the V projection runs in
    the 8-14us pre-exp window, also warming HAM), then the fqk chunks,
    then fv 6-7 and the host-pretransposed residual. wvT + q/k weights
    ride the parallel gpsimd queue.
  - block 0: per chunk [proj, qk_pair, 2x vproj, qk_pair]; the exp
    stream starts ~13us and never gaps (chunk DMA stays well ahead).
  - blocks 1-7: [qk_pair, av_task] per pair-slot; one 8-matmul AV task
    (0.88us) + one pair (0.21us) fits the 1.104us exp window with
    qk bufs=3 absorbing scheduling jitter.
  - AV tasks are a FIFO in itg-pair-major order (i0c0,i1c0,i0c1,... then
    i2/i3) so at most 2 avt accumulators are live (av_psum bufs=2);
    readiness gating (+2 pair margin) keeps the PE off far-future exp
    semaphores. The ~10-task steady backlog drains in the tail.
"""

import numpy as np

import concourse.bass as bass
import concourse.tile as tile
from concourse import bacc, mybir
from concourse.bass_utils import run_bass_kernel_spmd

C = 256
RD = 32
B = 4
N = 64 * 64  # 4096
NCORES = 8

IBLK = 512           # i-block size (query block)
NIB = N // IBLK      # 8
JT = 128             # j tile size
NJT = N // JT        # 32
ITPB = IBLK // 128   # 128-wide i-tiles per block = 4
VX = C + 1           # moving width of the AV matmul (values + ones column)

F32 = mybir.dt.float32
F32R = mybir.dt.float32r
BF16 = mybir.dt.bfloat16
F16 = mybir.dt.float16


def _build_bass():
    nc = bacc.Bacc(
        "TRN2",
        target_bir_lowering=False,
        debug=False,
        num_devices=NCORES,
    )

    feat_qk = nc.dram_tensor("feat_qk", [C, N], F16, kind="ExternalInput").ap()
    fv16 = nc.dram_tensor("fv16", [C, N], BF16, kind="ExternalInput").ap()
    # residual (feat_v + bv)^T, host-pretransposed: [p, m, c] = feat_v[c, m*128+p]+bv[c]
    fvbT16 = nc.dram_tensor("fvbT16", [128, N // 128, C], BF16, kind="ExternalInput").ap()
    # host-packed weights: [cin_inner=128, cin_outer=2, cout]
    wqkT = nc.dram_tensor("wqkT", [128, 2, 8 * RD], F16, kind="ExternalInput").ap()
    wvT = nc.dram_tensor("wvT", [128, 2, C], BF16, kind="ExternalInput").ap()
    bqk = nc.dram_tensor("bqk", [4 * RD, 2], F32, kind="ExternalInput").ap()
    # transposed output [i, c]; host flips back to [C, N]
    out = nc.dram_tensor("out_t", [N, C], F32, kind="ExternalOutput").ap()

    with tile.TileContext(nc) as tc:
        _kernel_body(nc, tc, feat_qk, fv16, fvbT16, wqkT, wvT, bqk, out)
    nc.compile()
    return nc


def _kernel_body(nc, tc, feat_qk, fv16, fvbT16, wqkT, wvT, bqk, out):
    Exp = mybir.ActivationFunctionType.Exp
    with (
        tc.tile_pool(name="singles", bufs=1) as singles,
        tc.tile_pool(name="work", bufs=4) as work,
        tc.tile_pool(name="opool", bufs=4) as opool,
        tc.tile_pool(name="upool", bufs=38) as upool,
        tc.tile_pool(name="qstage", bufs=4) as qstage,
        tc.tile_pool(name="vstage", bufs=4) as vstage,
        tc.tile_pool(name="qk_psum", bufs=3, space="PSUM") as qk_psum,
        tc.tile_pool(name="av_psum", bufs=2, space="PSUM") as av_psum,
    ):
        # ---- persistent SBUF ----
        # q/k: fp16, rows replicated 4x (row sets 0-31/32-63/64-95/96-127)
        q_sb = singles.tile([4 * RD, N], F16, tag="q")
        k_sb = singles.tile([4 * RD, N], F16, tag="k")
        # [V^T | ones] moving tiles: [j_inner, j_tile, VX] bf16 (col C = 1.0)
        vTx_sb = singles.tile([128, NJT, VX + 3], BF16, tag="vTx")
        # residual (feat_v + bv)^T tiles (host-pretransposed bf16)
        fvT_sb = singles.tile([128, N // 128, C], BF16, tag="fvT")

        wu_w = singles.tile([128, 128], BF16, tag="wu_w")
        wu_x = singles.tile([128, 512], BF16, tag="wu_x")
        dummy = singles.tile([128, 8], BF16, tag="dummy")

        # ---- DMA issue ----
        # gpsimd queue (parallel): wvT first (vproj needs it ~8us), then
        # q/k weights+biases (first projection needs them ~11us)
        wvT_sb = singles.tile([128, 2, C], BF16, tag="wvT")
        wqk_sb = singles.tile([128, 2, 8 * RD], F16, tag="wqk")
        bqk_sb = singles.tile([4 * RD, 2], F32, tag="bqk")
        wqT_sb = wqk_sb[:, :, 0 : 4 * RD]
        wkT_sb = wqk_sb[:, :, 4 * RD : 8 * RD]
        bq_sb = bqk_sb[:, 0:1]
        bk_sb = bqk_sb[:, 1:2]
        # warmup operands (gpsimd engine is otherwise idle)
        nc.gpsimd.memset(wu_w, 1.0)
        nc.gpsimd.memset(wu_x, 1.0)
        nc.gpsimd.memset(vTx_sb[:, :, C : C + 1], 1.0)
        # weights lead the sync queue (the gpsimd software-DGE queue is
        # far too slow for anything on the critical path): ~0.5MB, done
        # by ~8us, just ahead of the first projection

        # sync queue: fqk chunk 0 leads (it gates the first exp), fv
        # chunks interleave behind (they feed the vproj stream ~3 chunks
        # later), tail chunks and the residual last
        # 1024-col super-chunks: 2KB per-partition DMA lines (1KB lines
        # run at half DMA throughput and starved the exp stream)
        SC = 2 * IBLK
        fv_pn = fv16.rearrange("(a p) n -> p a n", a=2)
        fqk_pn = feat_qk.rearrange("(a p) n -> p a n", a=2)
        fv_sc = [None] * 4
        fqk_sc = [None] * 4

        def dma_fqk(s):
            fqk_st = qstage.tile([128, 2, SC], F16, tag="fqk_st")
            if s == 0:
                # 4 parallel descriptors: single early transfers are
                # engine-bound (~45GB/s); parallelism restores bandwidth
                for co in range(2):
                    for h in range(2):
                        nc.sync.dma_start(
                            out=fqk_st[:, co, h * IBLK : (h + 1) * IBLK],
                            in_=feat_qk[co * 128 : (co + 1) * 128,
                                        h * IBLK : (h + 1) * IBLK],
                        )
            else:
                nc.sync.dma_start(out=fqk_st, in_=fqk_pn[:, :, bass.ts(s, SC)])
            fqk_sc[s] = fqk_st

        def dma_fv(s):
            fv_st = vstage.tile([128, 2, SC], BF16, tag="fv_st")
            nc.sync.dma_start(out=fv_st, in_=fv_pn[:, :, bass.ts(s, SC)])
            fv_sc[s] = fv_st

        dma_fqk(0)
        nc.sync.dma_start(out=bqk_sb, in_=bqk)
        nc.sync.dma_start(out=wqk_sb, in_=wqkT)
        dma_fqk(1)
        nc.sync.dma_start(out=wvT_sb, in_=wvT)
        dma_fv(0)
        dma_fqk(2)
        dma_fv(1)
        dma_fqk(3)
        dma_fv(2)
        # residual tiles (host-pretransposed): 8 parallel chunks; the
        # first itg groups are needed by the epilogues from ~33us
        for m in range(2):
            nc.sync.dma_start(out=fvT_sb[:, 4 * m : 4 * m + 4, :],
                              in_=fvbT16[:, 4 * m : 4 * m + 4, :])
        dma_fv(3)
        for m in range(2, 8):
            nc.sync.dma_start(out=fvT_sb[:, 4 * m : 4 * m + 4, :],
                              in_=fvbT16[:, 4 * m : 4 * m + 4, :])

        # 512-col views in the original chunk indexing
        fqk_chunks = [fqk_sc[nb // 2][:, :, (nb % 2) * IBLK : (nb % 2 + 1) * IBLK]
                      for nb in range(NIB)]
        fv_chunks = [fv_sc[f // 2][:, :, (f % 2) * IBLK : (f % 2 + 1) * IBLK]
                     for f in range(NIB)]


        # scalar queue: tiny exp to pull the act table in (~2.7us) before
        # the first real exp
        nc.scalar.activation(out=dummy, in_=wu_x[:, 0:8], func=Exp)

        # PE warmup (HAM; bridges from ~7.4us to the vproj burst)
        for w in range(12):
            wup = av_psum.tile([128, 512], F32, tag="av", name="wup")
            nc.tensor.matmul(wup, wu_w, wu_x, start=True, stop=True)

        # ---- building blocks ----
        def proj_chunk(nb):
            ns = bass.ts(nb, IBLK)
            for (wT_sb, b_sb, dst) in (
                (wqT_sb, bq_sb, q_sb),
                (wkT_sb, bk_sb, k_sb),
            ):
                pp = av_psum.tile([128, 512], F32, tag="av", name="pp")
                for co in range(2):
                    nc.tensor.matmul(
                        pp,
                        wT_sb[:, co, :],
                        fqk_chunks[nb][:, co, :],
                        start=(co == 0),
                        stop=(co == 1),
                    )
                nc.vector.tensor_scalar_add(out=dst[:, ns], in0=pp, scalar1=b_sb)

        def qk_pair(nb, jp, u_list):
            # two j-tiles, concurrent matmuls on alternating PE row sets
            ns = bass.ts(nb, IBLK)
            r = jp % 2
            lp = qk_psum.tile([128, 2 * IBLK], F32, tag="qk")
            for h in range(2):
                jt = 2 * jp + h
                rows = slice(64 * r + 32 * h, 64 * r + 32 * h + 32)
                nc.tensor.matmul(
                    lp[:, h * IBLK : (h + 1) * IBLK],
                    k_sb[rows, bass.ts(jt, JT)],
                    q_sb[rows, ns],
                    start=True,
                    stop=True,
                    tile_position=(64 * r + 32 * h, 0),
                )
            ut = upool.tile([JT, 2 * IBLK], BF16, tag="u")
            nc.scalar.activation(out=ut, in_=lp, func=Exp)
            u_list.append(ut)

        def vproj_quad(g):
            # 4 j-tiles of the V projection in one qk-pool psum tile
            # (2 banks = same slot size as lp: keeps the pool rotation
            # uniform at [lp, lp, vpq] per chunk)
            vp = qk_psum.tile([128, 4, 256], F32, tag="qk", name="vp")
            for u in range(4):
                jt = 4 * g + u
                for co in range(2):
                    nc.tensor.matmul(
                        vp[:, u, :],
                        fv_chunks[jt // 4][:, co, bass.ts(jt % 4, JT)],
                        wvT_sb[:, co, :],
                        start=(co == 0),
                        stop=(co == 1),
                    )
            nc.vector.tensor_copy(
                out=vTx_sb[:, 4 * g : 4 * g + 4, 0:C], in_=vp
            )

        def av_epilogue(avt, itg):
            recip = work.tile([128, 1], F32, tag="recip")
            nc.vector.reciprocal(recip, avt[:, C : C + 1])
            o = opool.tile([128, C], F32, tag="o")
            nc.vector.tensor_scalar(
                out=o, in0=avt[:, 0:C], scalar1=recip, scalar2=None,
                op0=mybir.AluOpType.mult,
            )
            nc.vector.tensor_add(o, o, fvT_sb[:, itg, :])
            nc.sync.dma_start(out=out[bass.ts(itg, 128), :], in_=o)

        # ---- causal AV task FIFO ----
        # task = (u_list, itg, cch, ready): 8 matmuls accumulating avt[itg]
        # over j-tiles 8*cch..8*cch+7. ready = index of the last qk pair
        # whose exp produces those u tiles; popped only once that pair +2
        # margin is issued. itg-pair-major order keeps <=2 avt live.
        av_tasks = []
        avt_live = {}
        pairs_issued = [0]

        def push_block_tasks(nb, u_list):
            for itp in (0, 2):
                for cch in range(4):
                    ready = 16 * nb + 4 * cch + 3
                    for it in (itp, itp + 1):
                        av_tasks.append((u_list, nb * ITPB + it, cch, ready))

        def av_slot(margin=2):
            if not av_tasks or av_tasks[0][3] + margin >= pairs_issued[0]:
                return False
            u_list, itg, cch, _ = av_tasks.pop(0)
            if cch == 0:
                avt_live[itg] = av_psum.tile(
                    [128, VX + 3], F32, tag="av", name="avt"
                )
            avt = avt_live[itg]
            it = itg % ITPB
            for jt in range(8 * cch, 8 * cch + 8):
                nc.tensor.matmul(
                    avt[:, 0:VX],
                    u_list[jt // 2][
                        :,
                        (jt % 2) * IBLK + it * 128 : (jt % 2) * IBLK + it * 128 + 128,
                    ],
                    vTx_sb[:, jt, 0:VX],
                    start=(jt == 0),
                    stop=(jt == NJT - 1),
                )
            if cch == 3:
                av_epilogue(avt_live.pop(itg), itg)
            return True

        def issue_pair(nb, jp, u_list):
            qk_pair(nb, jp, u_list)
            pairs_issued[0] += 1

        # ---- block 0: V projection + per-chunk projection + QK/exp ----
        u_blocks = [[] for _ in range(NIB)]
        push_block_tasks(0, u_blocks[0])
        for nb in range(NIB):
            proj_chunk(nb)
            issue_pair(0, 2 * nb, u_blocks[0])
            issue_pair(0, 2 * nb + 1, u_blocks[0])
            if nb >= 1:
                vproj_quad(nb - 1)
        vproj_quad(7)
        for _ in range(4):
            av_slot()

        # ---- blocks 1..7: [qk_pair, av_task] per pair-slot ----
        for nb in range(1, NIB):
            push_block_tasks(nb, u_blocks[nb])
            for jp in range(0, NJT // 2, 2):
                issue_pair(nb, jp, u_blocks[nb])
                issue_pair(nb, jp + 1, u_blocks[nb])
                av_slot()
                av_slot()
        # tail: remaining AV backlog
        while av_tasks:
            if not av_slot(margin=-(10 ** 9)):
                raise AssertionError("av task FIFO stuck")


_NC_CACHE = None


def _get_nc():
    global _NC_CACHE
    if _NC_CACHE is None:
        _NC_CACHE = _build_bass()
    return _NC_CACHE


def _pack_qkT(w):
    # [RD, C] -> [128, 2, 4*RD] fp16: wT[p, co, r*RD+d] = w[d, co*128+p]
    wt = np.ascontiguousarray(w, dtype=np.float32).T.reshape(2, 128, RD)
    return np.ascontiguousarray(
        np.tile(wt, (1, 1, 4)).transpose(1, 0, 2).astype(np.float16)
    )


def _pack_vT(w, bf16):
    # [C, C] -> [128, 2, C] bf16: wvT[p, i, c] = w[c, i*128+p]
    wt = np.ascontiguousarray(w, dtype=np.float32).T.reshape(2, 128, C)
    return np.ascontiguousarray(wt.transpose(1, 0, 2).astype(bf16))


def _pack_fvbT(f, bv, bf16):
    # [C, N] + [C] -> [128, N//128, C] bf16: [p, m, c] = f[c, m*128+p]+bv[c]
    t = (f + bv[:, None]).T.reshape(N // 128, 128, C)
    return np.ascontiguousarray(t.transpose(1, 0, 2).astype(bf16))


def kernel(x_features, y_features, wqx, bqx, wkx, bkx, wvx, bvx,
           wqy, bqy, wky, bky, wvy, bvy):
    import ml_dtypes

    bf16 = ml_dtypes.bfloat16
    nc = _get_nc()

    def c(a):
        return np.ascontiguousarray(np.asarray(a), dtype=np.float32)

    def rep4(b):
        return np.ascontiguousarray(np.tile(c(b), 4)[:, None])

    in_maps = []
    for b in range(B):
        xf = c(x_features[b]).reshape(C, N)
        yf = c(y_features[b]).reshape(C, N)
        xf16 = np.ascontiguousarray(xf.astype(bf16))
        yf16 = np.ascontiguousarray(yf.astype(bf16))
        xfh = np.ascontiguousarray(xf.astype(np.float16))
        yfh = np.ascontiguousarray(yf.astype(np.float16))
        # core 2b: enhanced_x[b] — attention from y features, values from x
        in_maps.append({
            "feat_qk": yfh, "fv16": xf16,
            "fvbT16": _pack_fvbT(xf, c(bvx), bf16),
            "wqkT": np.concatenate([_pack_qkT(wqy), _pack_qkT(wky)], axis=2),
            "wvT": _pack_vT(wvx, bf16),
            "bqk": np.concatenate([rep4(bqy), rep4(bky)], axis=1),
        })
        # core 2b+1: enhanced_y[b] — attention from x features, values from y
        in_maps.append({
            "feat_qk": xfh, "fv16": yf16,
            "fvbT16": _pack_fvbT(yf, c(bvy), bf16),
            "wqkT": np.concatenate([_pack_qkT(wqx), _pack_qkT(wkx)], axis=2),
            "wvT": _pack_vT(wvy, bf16),
            "bqk": np.concatenate([rep4(bqx), rep4(bkx)], axis=1),
        })

    res = run_bass_kernel_spmd(nc, in_maps, core_ids=list(range(NCORES)))
    # out_t is [N, C]; flip back to [C, 64, 64]
    outs = [
        np.ascontiguousarray(r["out_t"].T).reshape(C, 64, 64)
        for r in res.results
    ]
    enhanced_x = np.stack(outs[0::2], axis=0)
    enhanced_y = np.stack(outs[1::2], axis=0)
    return enhanced_x, enhanced_y


# revision 16
# speedup vs baseline: 1.0202x; 1.0202x over previous
"""CMSA (cross-modal self-attention) Trainium2 Bass kernel, v16.

Problem: two feature maps x,y of [B=4, C=256, H=64, W=64]. Per sample:
  q_y,k_y = 1x1conv(y) -> [32, N]; v_x = 1x1conv(x) -> [256, N]  (N=4096)
  att_y = softmax(q_y^T k_y); enhanced_x = v_x @ att_y^T + x
  (and symmetrically x->y). Output: (enhanced_x, enhanced_y).

Sharding: 8 independent attention problems = (4 samples) x (2 directions),
one per NeuronCore, SPMD. Per-core kernel computes one full attention.

Kernel math (per core):
  L^T[j,i] = sum_d k[d,j] q[d,i]     (k-tile stationary fp16)
  U^T[j,i] = exp(L^T[j,i])           (unnormalized bf16)
  T[i, 0:256] = sum_j U^T[j,i] V^T[j,c]   "transposed AV": U^T-slice is the
  T[i, 256]   = sum_j U^T[j,i]             stationary operand, [V^T | ones]
                                           (257 cols) is the moving operand;
                                           denominator rides as column 256
  out^T[i,c] = T[i,c] / T[i,256] + (feat_v^T[i,c] + bv[c])

v8 schedule. The ACT exp stream (1.104us per [128,1024] tile, 128 tiles
= 141us) is the pacer; the PE carries ~9% more work than ACT, so the
structural excess surfaces only in the tail:
  - DMA: sync queue sends fv chunks 0-5 first (the V projection runs in
    the 8-14us pre-exp window, also warming HAM), then the fqk chunks,
    then fv 6-7 and the host-pretransposed residual. wvT + q/k weights
    ride the parallel gpsimd queue.
  - block 0: per chunk [proj, qk_pair, 2x vproj, qk_pair]; the exp
    stream starts ~13us and never gaps (chunk DMA stays well ahead).
  - blocks 1-7: [qk_pair, av_task] per pair-slot; one 8-matmul AV task
    (0.88us) + one pair (0.21us) fits the 1.104us exp window with
    qk bufs=3 absorbing scheduling jitter.
  - AV tasks are a FIFO in itg-pair-major order (i0c0,i1c0,i0c1,... then
    i2/i3) so at most 2 avt accumulators are live (av_psum bufs=2);
    readiness gating (+2 pair margin) keeps the PE off far-future exp
    semaphores. The ~10-task steady backlog drains in the tail.
"""

import numpy as np

import concourse.bass as bass
import concourse.tile as tile
from concourse import bacc, mybir
from concourse.bass_utils import run_bass_kernel_spmd

C = 256
RD = 32
B = 4
N = 64 * 64  # 4096
NCORES = 8

IBLK = 512           # i-block size (query block)
NIB = N // IBLK      # 8
JT = 128             # j tile size
NJT = N // JT        # 32
ITPB = IBLK // 128   # 128-wide i-tiles per block = 4
VX = C + 1           # moving width of the AV matmul (values + ones column)

F32 = mybir.dt.float32
F32R = mybir.dt.float32r
BF16 = mybir.dt.bfloat16
F16 = mybir.dt.float16


def _build_bass():
    nc = bacc.Bacc(
        "TRN2",
        target_bir_lowering=False,
        debug=False,
        num_devices=NCORES,
    )

    feat_qk = nc.dram_tensor("feat_qk", [C, N], F16, kind="ExternalInput").ap()
    fv16 = nc.dram_tensor("fv16", [C, N], BF16, kind="ExternalInput").ap()
    # residual (feat_v + bv)^T, host-pretransposed: [p, m, c] = feat_v[c, m*128+p]+bv[c]
    fvbT16 = nc.dram_tensor("fvbT16", [128, N // 128, C], BF16, kind="ExternalInput").ap()
    # host-packed weights: [cin_inner=128, cin_outer=2, cout]
    wqkT = nc.dram_tensor("wqkT", [128, 2, 8 * RD], F16, kind="ExternalInput").ap()
    wvT = nc.dram_tensor("wvT", [128, 2, C], BF16, kind="ExternalInput").ap()
    bqk = nc.dram_tensor("bqk", [4 * RD, 2], F32, kind="ExternalInput").ap()
    # transposed output [i, c]; host flips back to [C, N]
    out = nc.dram_tensor("out_t", [N, C], F32, kind="ExternalOutput").ap()

    with tile.TileContext(nc) as tc:
        _kernel_body(nc, tc, feat_qk, fv16, fvbT16, wqkT, wvT, bqk, out)
    nc.compile()
    return nc


def _kernel_body(nc, tc, feat_qk, fv16, fvbT16, wqkT, wvT, bqk, out):
    Exp = mybir.ActivationFunctionType.Exp
    with (
        tc.tile_pool(name="singles", bufs=1) as singles,
        tc.tile_pool(name="work", bufs=4) as work,
        tc.tile_pool(name="opool", bufs=4) as opool,
        tc.tile_pool(name="upool", bufs=38) as upool,
        tc.tile_pool(name="qstage", bufs=4) as qstage,
        tc.tile_pool(name="vstage", bufs=4) as vstage,
        tc.tile_pool(name="qk_psum", bufs=3, space="PSUM") as qk_psum,
        tc.tile_pool(name="av_psum", bufs=2, space="PSUM") as av_psum,
    ):
        # ---- persistent SBUF ----
        # q/k: fp16, rows replicated 4x (row sets 0-31/32-63/64-95/96-127)
        q_sb = singles.tile([4 * RD, N], F16, tag="q")
        k_sb = singles.tile([4 * RD, N], F16, tag="k")
        # [V^T | ones] moving tiles: [j_inner, j_tile, VX] bf16 (col C = 1.0)
        vTx_sb = singles.tile([128, NJT, VX + 3], BF16, tag="vTx")
        # residual (feat_v + bv)^T tiles (host-pretransposed bf16)
        fvT_sb = singles.tile([128, N // 128, C], BF16, tag="fvT")

        wu_w = singles.tile([128, 128], BF16, tag="wu_w")
        wu_x = singles.tile([128, 512], BF16, tag="wu_x")
        dummy = singles.tile([128, 8], BF16, tag="dummy")

        # ---- DMA issue ----
        # gpsimd queue (parallel): wvT first (vproj needs it ~8us), then
        # q/k weights+biases (first projection needs them ~11us)
        wvT_sb = singles.tile([128, 2, C], BF16, tag="wvT")
        wqk_sb = singles.tile([128, 2, 8 * RD], F16, tag="wqk")
        bqk_sb = singles.tile([4 * RD, 2], F32, tag="bqk")
        wqT_sb = wqk_sb[:, :, 0 : 4 * RD]
        wkT_sb = wqk_sb[:, :, 4 * RD : 8 * RD]
        bq_sb = bqk_sb[:, 0:1]
        bk_sb = bqk_sb[:, 1:2]
        # warmup operands (gpsimd engine is otherwise idle)
        nc.gpsimd.memset(wu_w, 1.0)
        nc.gpsimd.memset(wu_x, 1.0)
        nc.gpsimd.memset(vTx_sb[:, :, C : C + 1], 1.0)
        # weights lead the sync queue (the gpsimd software-DGE queue is
        # far too slow for anything on the critical path): ~0.5MB, done
        # by ~8us, just ahead of the first projection

        # sync queue: fqk chunk 0 leads (it gates the first exp), fv
        # chunks interleave behind (they feed the vproj stream ~3 chunks
        # later), tail chunks and the residual last
        # 1024-col super-chunks: 2KB per-partition DMA lines (1KB lines
        # run at half DMA throughput and starved the exp stream)
        SC = 2 * IBLK
        fv_pn = fv16.rearrange("(a p) n -> p a n", a=2)
        fqk_pn = feat_qk.rearrange("(a p) n -> p a n", a=2)
        fv_sc = [None] * 4
        fqk_sc = [None] * 4

        def dma_fqk(s):
            fqk_st = qstage.tile([128, 2, SC], F16, tag="fqk_st")
            if s == 0:
                # 4 parallel descriptors: single early transfers are
                # engine-bound (~45GB/s); parallelism restores bandwidth
                for co in range(2):
                    for h in range(2):
                        nc.sync.dma_start(
                            out=fqk_st[:, co, h * IBLK : (h + 1) * IBLK],
                            in_=feat_qk[co * 128 : (co + 1) * 128,
                                        h * IBLK : (h + 1) * IBLK],
                        )
            else:
                nc.sync.dma_start(out=fqk_st, in_=fqk_pn[:, :, bass.ts(s, SC)])
            fqk_sc[s] = fqk_st

        def dma_fv(s):
            fv_st = vstage.tile([128, 2, SC], BF16, tag="fv_st")
            nc.sync.dma_start(out=fv_st, in_=fv_pn[:, :, bass.ts(s, SC)])
            fv_sc[s] = fv_st

        dma_fqk(0)
        nc.sync.dma_start(out=bqk_sb, in_=bqk)
        nc.sync.dma_start(out=wqk_sb, in_=wqkT)
        dma_fqk(1)
        nc.sync.dma_start(out=wvT_sb, in_=wvT)
        dma_fv(0)
        dma_fqk(2)
        dma_fv(1)
        dma_fqk(3)
        dma_fv(2)
        # residual tiles (host-pretransposed): 8 parallel chunks; the
        # first itg groups are needed by the epilogues from ~33us
        for m in range(2):
            nc.sync.dma_start(out=fvT_sb[:, 4 * m : 4 * m + 4, :],
                              in_=fvbT16[:, 4 * m : 4 * m + 4, :])
        dma_fv(3)
        for m in range(2, 8):
            nc.sync.dma_start(out=fvT_sb[:, 4 * m : 4 * m + 4, :],
                              in_=fvbT16[:, 4 * m : 4 * m + 4, :])

        # 512-col views in the original chunk indexing
        fqk_chunks = [fqk_sc[nb // 2][:, :, (nb % 2) * IBLK : (nb % 2 + 1) * IBLK]
                      for nb in range(NIB)]
        fv_chunks = [fv_sc[f // 2][:, :, (f % 2) * IBLK : (f % 2 + 1) * IBLK]
                     for f in range(NIB)]


        # scalar queue: tiny exp to pull the act table in (~2.7us) before
        # the first real exp
        nc.scalar.activation(out=dummy, in_=wu_x[:, 0:8], func=Exp)

        # PE warmup (HAM; bridges from ~7.4us to the vproj burst)
        for w in range(12):
            wup = av_psum.tile([128, 512], F32, tag="av", name="wup")
            nc.tensor.matmul(wup, wu_w, wu_x, start=True, stop=True)

        # ---- building blocks ----
        def proj_chunk(nb):
            ns = bass.ts(nb, IBLK)
            for (wT_sb, b_sb, dst) in (
                (wqT_sb, bq_sb, q_sb),
                (wkT_sb, bk_sb, k_sb),
            ):
                pp = av_psum.tile([128, 512], F32, tag="av", name="pp")
                for co in range(2):
                    nc.tensor.matmul(
                        pp,
                        wT_sb[:, co, :],
                        fqk_chunks[nb][:, co, :],
                        start=(co == 0),
                        stop=(co == 1),
                    )
                nc.vector.tensor_scalar_add(out=dst[:, ns], in0=pp, scalar1=b_sb)

        def qk_pair(nb, jp, u_list):
            # two j-tiles, concurrent matmuls on alternating PE row sets
            ns = bass.ts(nb, IBLK)
            r = jp % 2
            lp = qk_psum.tile([128, 2 * IBLK], F32, tag="qk")
            for h in range(2):
                jt = 2 * jp + h
                rows = slice(64 * r + 32 * h, 64 * r + 32 * h + 32)
                nc.tensor.matmul(
                    lp[:, h * IBLK : (h + 1) * IBLK],
                    k_sb[rows, bass.ts(jt, JT)],
                    q_sb[rows, ns],
                    start=True,
                    stop=True,
                    tile_position=(64 * r + 32 * h, 0),
                )
            ut = upool.tile([JT, 2 * IBLK], BF16, tag="u")
            nc.scalar.activation(out=ut, in_=lp, func=Exp)
            u_list.append(ut)

        def vproj_pair(k):
            # 2 j-tiles of the V projection in one qk-pool psum tile
            vp = qk_psum.tile([128, 2, 256], F32, tag="qk", name="vp")
            for u in range(2):
                jt = 2 * k + u
                for co in range(2):
                    nc.tensor.matmul(
                        vp[:, u, :],
                        fv_chunks[jt // 4][:, co, bass.ts(jt % 4, JT)],
                        wvT_sb[:, co, :],
                        start=(co == 0),
                        stop=(co == 1),
                    )
            nc.vector.tensor_copy(
                out=vTx_sb[:, 2 * k : 2 * k + 2, 0:C], in_=vp
            )

        def av_epilogue(avt, itg):
            recip = work.tile([128, 1], F32, tag="recip")
            nc.vector.reciprocal(recip, avt[:, C : C + 1])
            o = opool.tile([128, C], F32, tag="o")
            nc.vector.tensor_scalar(
                out=o, in0=avt[:, 0:C], scalar1=recip, scalar2=None,
                op0=mybir.AluOpType.mult,
            )
            nc.vector.tensor_add(o, o, fvT_sb[:, itg, :])
            nc.sync.dma_start(out=out[bass.ts(itg, 128), :], in_=o)

        # ---- causal AV task FIFO ----
        # task = (u_list, itg, cch, ready): 8 matmuls accumulating avt[itg]
        # over j-tiles 8*cch..8*cch+7. ready = index of the last qk pair
        # whose exp produces those u tiles; popped only once that pair +2
        # margin is issued. itg-pair-major order keeps <=2 avt live.
        av_tasks = []
        avt_live = {}
        pairs_issued = [0]

        def push_block_tasks(nb, u_list):
            for itp in (0, 2):
                for cch in range(4):
                    ready = 16 * nb + 4 * cch + 3
                    for it in (itp, itp + 1):
                        av_tasks.append((u_list, nb * ITPB + it, cch, ready))

        def av_slot(margin=2):
            if not av_tasks or av_tasks[0][3] + margin >= pairs_issued[0]:
                return False
            u_list, itg, cch, _ = av_tasks.pop(0)
            if cch == 0:
                avt_live[itg] = av_psum.tile(
                    [128, VX + 3], F32, tag="av", name="avt"
                )
            avt = avt_live[itg]
            it = itg % ITPB
            for jt in range(8 * cch, 8 * cch + 8):
                nc.tensor.matmul(
                    avt[:, 0:VX],
                    u_list[jt // 2][
                        :,
                        (jt % 2) * IBLK + it * 128 : (jt % 2) * IBLK + it * 128 + 128,
                    ],
                    vTx_sb[:, jt, 0:VX],
                    start=(jt == 0),
                    stop=(jt == NJT - 1),
                )
            if cch == 3:
                av_epilogue(avt_live.pop(itg), itg)
            return True

        def issue_pair(nb, jp, u_list):
            qk_pair(nb, jp, u_list)
            pairs_issued[0] += 1

        # ---- block 0: V projection + per-chunk projection + QK/exp ----
        u_blocks = [[] for _ in range(NIB)]
        push_block_tasks(0, u_blocks[0])
        for nb in range(NIB):
            proj_chunk(nb)
            issue_pair(0, 2 * nb, u_blocks[0])
            issue_pair(0, 2 * nb + 1, u_blocks[0])
            for k in range(max(2 * nb - 2, 0), 2 * nb):
                vproj_pair(k)
        for k in range(14, 16):
            vproj_pair(k)
        for _ in range(4):
            av_slot()

        # ---- blocks 1..7: [qk_pair, av_task] per pair-slot ----
        for nb in range(1, NIB):
            push_block_tasks(nb, u_blocks[nb])
            for jp in range(0, NJT // 2, 2):
                issue_pair(nb, jp, u_blocks[nb])
                issue_pair(nb, jp + 1, u_blocks[nb])
                av_slot()
                av_slot()
        # tail: remaining AV backlog
        while av_tasks:
            if not av_slot(margin=-(10 ** 9)):
                raise AssertionError("av task FIFO stuck")


_NC_CACHE = None


def _get_nc():
    global _NC_CACHE
    if _NC_CACHE is None:
        _NC_CACHE = _build_bass()
    return _NC_CACHE


def _pack_qkT(w):
    # [RD, C] -> [128, 2, 4*RD] fp16: wT[p, co, r*RD+d] = w[d, co*128+p]
    wt = np.ascontiguousarray(w, dtype=np.float32).T.reshape(2, 128, RD)
    return np.ascontiguousarray(
        np.tile(wt, (1, 1, 4)).transpose(1, 0, 2).astype(np.float16)
    )


def _pack_vT(w, bf16):
    # [C, C] -> [128, 2, C] bf16: wvT[p, i, c] = w[c, i*128+p]
    wt = np.ascontiguousarray(w, dtype=np.float32).T.reshape(2, 128, C)
    return np.ascontiguousarray(wt.transpose(1, 0, 2).astype(bf16))


def _pack_fvbT(f, bv, bf16):
    # [C, N] + [C] -> [128, N//128, C] bf16: [p, m, c] = f[c, m*128+p]+bv[c]
    t = (f + bv[:, None]).T.reshape(N // 128, 128, C)
    return np.ascontiguousarray(t.transpose(1, 0, 2).astype(bf16))


def kernel(x_features, y_features, wqx, bqx, wkx, bkx, wvx, bvx,
           wqy, bqy, wky, bky, wvy, bvy):
    import ml_dtypes

    bf16 = ml_dtypes.bfloat16
    nc = _get_nc()

    def c(a):
        return np.ascontiguousarray(np.asarray(a), dtype=np.float32)

    def rep4(b):
        return np.ascontiguousarray(np.tile(c(b), 4)[:, None])

    in_maps = []
    for b in range(B):
        xf = c(x_features[b]).reshape(C, N)
        yf = c(y_features[b]).reshape(C, N)
        xf16 = np.ascontiguousarray(xf.astype(bf16))
        yf16 = np.ascontiguousarray(yf.astype(bf16))
        xfh = np.ascontiguousarray(xf.astype(np.float16))
        yfh = np.ascontiguousarray(yf.astype(np.float16))
        # core 2b: enhanced_x[b] — attention from y features, values from x
        in_maps.append({
            "feat_qk": yfh, "fv16": xf16,
            "fvbT16": _pack_fvbT(xf, c(bvx), bf16),
            "wqkT": np.concatenate([_pack_qkT(wqy), _pack_qkT(wky)], axis=2),
            "wvT": _pack_vT(wvx, bf16),
            "bqk": np.concatenate([rep4(bqy), rep4(bky)], axis=1),
        })
        # core 2b+1: enhanced_y[b] — attention from x features, values from y
        in_maps.append({
            "feat_qk": xfh, "fv16": yf16,
            "fvbT16": _pack_fvbT(yf, c(bvy), bf16),
            "wqkT": np.concatenate([_pack_qkT(wqx), _pack_qkT(wkx)], axis=2),
            "wvT": _pack_vT(wvy, bf16),
            "bqk": np.concatenate([rep4(bqx), rep4(bkx)], axis=1),
        })

    res = run_bass_kernel_spmd(nc, in_maps, core_ids=list(range(NCORES)))
    # out_t is [N, C]; flip back to [C, 64, 64]
    outs = [
        np.ascontiguousarray(r["out_t"].T).reshape(C, 64, 64)
        for r in res.results
    ]
    enhanced_x = np.stack(outs[0::2], axis=0)
    enhanced_y = np.stack(outs[1::2], axis=0)
    return enhanced_x, enhanced_y
